# revision 6
# baseline (speedup 1.0000x reference)
"""AttentionRNN Trainium2 kernel.

Model (per batch element, sizes hardcoded):
    e = emb[x]                                          # (T=256, E=64) gather
    h_t = tanh(W_ih e_t + b_ih + W_hh h_{t-1} + b_hh)   # serial scan, H=128
    qk = Wq @ rnn_out ; kk = Wk @ rnn_out               # (H, T)
    score[t,s] = v . tanh(qk[:,t] + kk[:,s]), causal s<=t
    attn = softmax_s(score) ; ctx[:,t] = sum_s attn[t,s] rnn_out[:,s]
    logits = fc_W @ [rnn_out; ctx] + fc_b               # (V=96, T)

Sharding: pure data-parallel — core b computes batch element b end-to-end.
Host only reshapes/transposes weights (layout prep) and stacks outputs.

Device design notes:
  * rnn_out, qk, kk, U live as [H=128 partitions, T=256 free].
  * Scan: W_hh.T stays lhsT; per step one matmul (N=1) + one ACT
    tanh whose per-partition bias is the precomputed U column.
  * Pair tensor A[h, (t_chunk, s)] = qk[:,t]+kk[:,s] is built by a single
    DVE tensor_tensor add using broadcast access patterns, tanh'd on ACT.
  * v-dot: per query, matmul with the pair slice as the *stationary*
    operand (lhsT [h, s_tile]) and v as the moving one -> scores land
    directly as [s, t] PSUM tiles (the lhsT free dim becomes partitions).
  * Softmax skips max subtraction: |score| <= ||v||_1 ~ 9, exp is safe in
    fp32 and matches the reference's stabilized softmax to rounding.
    Causal mask = zero-fill after exp (affine_select), so masked keys drop
    out of both the denominator and the context sum.
  * Denominator: ones-vector matmul over s -> [1, T] psum, copied (one
    lane, 256 elements) to SBUF, moved to [t, 1] via a K=1 matmul, then
    reciprocal. Context is computed transposed (ctxT[t, h], attn as lhsT)
    so the normalization is a per-partition tensor_scalar multiply.
"""

import numpy as np

import concourse.bacc as bacc
import concourse.bass as bass
import concourse.tile as tile
from concourse import mybir
from concourse.bass_utils import run_bass_kernel_spmd
from concourse.masks import make_identity

B, T, V, E, H = 8, 256, 96, 64, 128
N_CORES = 8
FP = mybir.dt.float32
AF = mybir.ActivationFunctionType
OP = mybir.AluOpType
AX = mybir.AxisListType

CQ = 16  # queries per pair-chunk


def _bcast_mid(ap2d, n):
    """[P, s] -> [P, (broadcast n), s]"""
    return bass.AP(tensor=ap2d.tensor, offset=ap2d.offset,
                   ap=[ap2d.ap[0], [0, n], ap2d.ap[1]])


def _bcast_inner(ap2d, n):
    """[P, c] -> [P, c, (broadcast n)]"""
    return bass.AP(tensor=ap2d.tensor, offset=ap2d.offset,
                   ap=[ap2d.ap[0], ap2d.ap[1], [0, n]])


def build_kernel():
    nc = bacc.Bacc()

    # Per-core inputs (weights replicated, x/h0 sharded by batch).
    x_ids = nc.dram_tensor("x_ids", [T, 1], mybir.dt.int32, kind="ExternalInput")
    h0c = nc.dram_tensor("h0c", [H, 1], FP, kind="ExternalInput")
    emb = nc.dram_tensor("emb", [V, E], FP, kind="ExternalInput")
    w_ihT = nc.dram_tensor("w_ihT", [E, H], FP, kind="ExternalInput")
    w_hhT = nc.dram_tensor("w_hhT", [H, H], FP, kind="ExternalInput")
    wqT = nc.dram_tensor("wqT", [H, H], FP, kind="ExternalInput")
    wkT = nc.dram_tensor("wkT", [H, H], FP, kind="ExternalInput")
    v_at = nc.dram_tensor("v_at", [H, 1], FP, kind="ExternalInput")
    fc_w1T = nc.dram_tensor("fc_w1T", [H, V], FP, kind="ExternalInput")
    fc_w2T = nc.dram_tensor("fc_w2T", [H, V], FP, kind="ExternalInput")
    fc_bd = nc.dram_tensor("fc_bd", [V, 1], FP, kind="ExternalInput")
    b_sum = nc.dram_tensor("b_sum", [H, 1], FP, kind="ExternalInput")

    logits_vT = nc.dram_tensor("logits_vT", [V, T], FP, kind="ExternalOutput")
    h_last = nc.dram_tensor("h_last", [H, 1], FP, kind="ExternalOutput")

    with tile.TileContext(nc) as tc:
        with (
            tc.tile_pool(name="consts", bufs=1) as consts,
            tc.tile_pool(name="work", bufs=1) as work,
            tc.tile_pool(name="pairs", bufs=2) as pairs,
        ):
            # ---- load constants ----
            ident = consts.tile([128, 128], FP)
            make_identity(nc, ident[:])
            ones_s = consts.tile([H, 1], FP)
            nc.vector.memset(ones_s[:], 1.0)
            w_ihT_s = consts.tile([E, H], FP)
            nc.sync.dma_start(out=w_ihT_s[:], in_=w_ihT[:])
            w_hhT_s = consts.tile([H, H], FP)
            nc.sync.dma_start(out=w_hhT_s[:], in_=w_hhT[:])
            wqT_s = consts.tile([H, H], FP)
            nc.sync.dma_start(out=wqT_s[:], in_=wqT[:])
            wkT_s = consts.tile([H, H], FP)
            nc.sync.dma_start(out=wkT_s[:], in_=wkT[:])
            v_s = consts.tile([H, 1], FP)
            nc.sync.dma_start(out=v_s[:], in_=v_at[:])
            fc_w1T_s = consts.tile([H, V], FP)
            nc.sync.dma_start(out=fc_w1T_s[:], in_=fc_w1T[:])
            fc_w2T_s = consts.tile([H, V], FP)
            nc.sync.dma_start(out=fc_w2T_s[:], in_=fc_w2T[:])
            fc_b_s = consts.tile([V, 1], FP)
            nc.sync.dma_start(out=fc_b_s[:], in_=fc_bd[:])
            b_sum_s = consts.tile([H, 1], FP)
            nc.sync.dma_start(out=b_sum_s[:], in_=b_sum[:])
            h0_s = consts.tile([H, 1], FP)
            nc.sync.dma_start(out=h0_s[:], in_=h0c[:])
            idx0 = consts.tile([128, 1], mybir.dt.int32)
            nc.sync.dma_start(out=idx0[:], in_=x_ids[0:128, :])
            idx1 = consts.tile([128, 1], mybir.dt.int32)
            nc.sync.dma_start(out=idx1[:], in_=x_ids[128:256, :])

            # ---- embedding gather + transpose to eT [E, T] ----
            eT = work.tile([E, T], FP)
            e0 = work.tile([128, E], FP)
            nc.gpsimd.indirect_dma_start(
                out=e0[:], out_offset=None, in_=emb[:],
                in_offset=bass.IndirectOffsetOnAxis(ap=idx0[:, :1], axis=0))
            e1 = work.tile([128, E], FP)
            nc.gpsimd.indirect_dma_start(
                out=e1[:], out_offset=None, in_=emb[:],
                in_offset=bass.IndirectOffsetOnAxis(ap=idx1[:, :1], axis=0))

            u_sb = work.tile([H, T], FP)
            rnn = work.tile([H, T], FP)

            with tc.tile_pool(name="ps1", bufs=2, space="PSUM") as ps1:
                eT_ps0 = ps1.tile([E, 128], FP, tag="eT_ps")
                nc.tensor.transpose(out=eT_ps0[:], in_=e0[:], identity=ident[:])
                nc.vector.tensor_copy(out=eT[:, 0:128], in_=eT_ps0[:])
                eT_ps1 = ps1.tile([E, 128], FP, tag="eT_ps")
                nc.tensor.transpose(out=eT_ps1[:], in_=e1[:], identity=ident[:])
                nc.vector.tensor_copy(out=eT[:, 128:256], in_=eT_ps1[:])

                # ---- U = W_ih @ e + (b_ih + b_hh), shape [H, T] ----
                u_ps = ps1.tile([H, T], FP, tag="u_ps")
                nc.tensor.matmul(out=u_ps[:], lhsT=w_ihT_s[:], rhs=eT[:],
                                 start=True, stop=True)
                nc.scalar.activation(out=u_sb[:], in_=u_ps[:], func=AF.Identity,
                                     bias=b_sum_s[:, 0:1], scale=1.0)

                # ---- serial RNN scan ----
                hprev = h0_s[:, 0:1]
                for t in range(T):
                    g_ps = ps1.tile([H, 1], FP, tag="g_ps")
                    nc.tensor.matmul(out=g_ps[:], lhsT=w_hhT_s[:], rhs=hprev,
                                     start=True, stop=True)
                    nc.scalar.activation(out=rnn[:, t:t + 1], in_=g_ps[:],
                                         func=AF.Tanh, bias=u_sb[:, t:t + 1],
                                         scale=1.0)
                    hprev = rnn[:, t:t + 1]

            nc.sync.dma_start(out=h_last[:], in_=rnn[:, T - 1:T])

            # ---- qk / kk [H, T] ----
            qk = work.tile([H, T], FP)
            kk = work.tile([H, T], FP)
            with tc.tile_pool(name="ps2", bufs=1, space="PSUM") as ps2:
                qk_ps = ps2.tile([H, T], FP, tag="qk_ps")
                nc.tensor.matmul(out=qk_ps[:], lhsT=wqT_s[:], rhs=rnn[:],
                                 start=True, stop=True)
                nc.vector.tensor_copy(out=qk[:], in_=qk_ps[:])
                kk_ps = ps2.tile([H, T], FP, tag="kk_ps")
                nc.tensor.matmul(out=kk_ps[:], lhsT=wkT_s[:], rhs=rnn[:],
                                 start=True, stop=True)
                nc.vector.tensor_copy(out=kk[:], in_=kk_ps[:])

            # ---- attention: scores in [s, t] psum tiles, exp, mask, ctx ----
            # att[tb][sb]: attention weights exp(score) for queries
            # t in [tb*128,(tb+1)*128), keys s in [sb*128,(sb+1)*128).
            att = {k: work.tile([128, 128], FP, tag=f"att{k[0]}{k[1]}",
                                name=f"att{k[0]}{k[1]}")
                   for k in ((0, 0), (1, 0), (1, 1))}
            rnnT0 = work.tile([128, H], FP)
            rnnT1 = work.tile([128, H], FP)
            ctx = work.tile([H, T], FP)
            den_sb = work.tile([1, T], FP)
            rdenT = work.tile([128, 2], FP)
            ctxT0 = work.tile([128, H], FP)
            ctxT1 = work.tile([128, H], FP)

            with tc.tile_pool(name="ps3", bufs=1, space="PSUM") as ps3:
                # rnnT tiles (needed for ctx): rnnT[sb] = rnn[:, sb].T
                tpa = ps3.tile([128, 128], FP, tag="mm", bufs=2)
                nc.tensor.transpose(out=tpa[:], in_=rnn[:, 0:128],
                                    identity=ident[:])
                nc.vector.tensor_copy(out=rnnT0[:], in_=tpa[:])
                tpb = ps3.tile([128, 128], FP, tag="mm", bufs=2)
                nc.tensor.transpose(out=tpb[:], in_=rnn[:, 128:256],
                                    identity=ident[:])
                nc.vector.tensor_copy(out=rnnT1[:], in_=tpb[:])

                for tb in range(2):
                    s_len = (tb + 1) * 128
                    st_ps = [ps3.tile([128, 128], FP, tag="st", bufs=2,
                                      name=f"st{tb}{sb}")
                             for sb in range(tb + 1)]
                    for c in range(128 // CQ):
                        q0 = tb * 128 + c * CQ
                        pair = pairs.tile([128, CQ, s_len], FP, tag="pair")
                        nc.vector.tensor_tensor(
                            out=pair[:],
                            in0=_bcast_mid(kk[:, 0:s_len], CQ),
                            in1=_bcast_inner(qk[:, q0:q0 + CQ], s_len),
                            op=OP.add)
                        nc.scalar.activation(out=pair[:], in_=pair[:],
                                             func=AF.Tanh)
                        for j in range(CQ):
                            tcol = c * CQ + j
                            for sb in range(tb + 1):
                                nc.tensor.matmul(
                                    out=st_ps[sb][:, tcol:tcol + 1],
                                    lhsT=pair[:, j, sb * 128:(sb + 1) * 128],
                                    rhs=v_s[:],
                                    start=True, stop=True)
                    # exp (no max subtraction; |score| <= ||v||_1 ~ 9)
                    # + causal zero-fill on the diagonal block
                    for sb in range(tb + 1):
                        a_t = att[(tb, sb)]
                        nc.scalar.activation(out=a_t[:], in_=st_ps[sb][:],
                                             func=AF.Exp)
                        if sb == tb:  # keep where t - s >= 0
                            nc.gpsimd.affine_select(
                                out=a_t[:], in_=a_t[:], pattern=[[1, 128]],
                                compare_op=OP.is_ge, fill=0.0,
                                base=0, channel_multiplier=-1)

                # denominator den[t] = sum_s att[s, t]  -> [1, T] psum
                den_ps = ps3.tile([1, T], FP, tag="den")
                for tb in range(2):
                    for sb in range(tb + 1):
                        nc.tensor.matmul(
                            out=den_ps[:, tb * 128:(tb + 1) * 128],
                            lhsT=ones_s[:], rhs=att[(tb, sb)][:],
                            start=(sb == 0), stop=(sb == tb))
                nc.vector.tensor_copy(out=den_sb[:], in_=den_ps[:])

                # denT via K=1 matmuls: out[t, 0] = den[0, t]
                denT_ps = ps3.tile([128, 2], FP, tag="mm", bufs=2)
                for tb in range(2):
                    nc.tensor.matmul(out=denT_ps[:, tb:tb + 1],
                                     lhsT=den_sb[:, tb * 128:(tb + 1) * 128],
                                     rhs=ones_s[0:1, 0:1],
                                     start=True, stop=True)
                nc.vector.reciprocal(out=rdenT[:], in_=denT_ps[:])

                # ctxT[t, h] = sum_s att[s, t] * rnnT[s, h], then scale by 1/den
                for tb, ctxT in ((0, ctxT0), (1, ctxT1)):
                    cps = ps3.tile([128, H], FP, tag="mm", bufs=2,
                                   name=f"cps{tb}")
                    for sb in range(tb + 1):
                        nc.tensor.matmul(
                            out=cps[:],
                            lhsT=att[(tb, sb)][:],
                            rhs=(rnnT0 if sb == 0 else rnnT1)[:],
                            start=(sb == 0), stop=(sb == tb))
                    nc.vector.tensor_scalar_mul(out=ctxT[:], in0=cps[:],
                                                scalar1=rdenT[:, tb:tb + 1])

                # ctx[h, t] = ctxT.T
                for tb, ctxT in ((0, ctxT0), (1, ctxT1)):
                    ctp = ps3.tile([H, 128], FP, tag="mm", bufs=2,
                                   name=f"ctp{tb}")
                    nc.tensor.transpose(out=ctp[:], in_=ctxT[:],
                                        identity=ident[:])
                    nc.vector.tensor_copy(out=ctx[:, tb * 128:(tb + 1) * 128],
                                          in_=ctp[:])

                # ---- FC: logits = fc_W1 @ rnn + fc_W2 @ ctx + fc_b ----
                lg_ps = ps3.tile([V, T], FP, tag="lg")
                nc.tensor.matmul(out=lg_ps[:], lhsT=fc_w1T_s[:], rhs=rnn[:],
                                 start=True, stop=False)
                nc.tensor.matmul(out=lg_ps[:], lhsT=fc_w2T_s[:], rhs=ctx[:],
                                 start=False, stop=True)
                lg = work.tile([V, T], FP)
                nc.scalar.activation(out=lg[:], in_=lg_ps[:], func=AF.Identity,
                                     bias=fc_b_s[:, 0:1], scale=1.0)
                nc.sync.dma_start(out=logits_vT[:], in_=lg[:])

    nc.compile()
    return nc


_NC = None


def _get_nc():
    global _NC
    if _NC is None:
        _NC = build_kernel()
    return _NC


def make_in_maps(inputs):
    x = np.asarray(inputs["x"]).astype(np.int32)            # (B, T)
    h0 = np.asarray(inputs["h0"], dtype=np.float32)         # (1, B, H)
    emb = np.ascontiguousarray(np.asarray(inputs["emb"], dtype=np.float32))
    w_ih = np.asarray(inputs["W_ih"], dtype=np.float32)     # (H, E)
    w_hh = np.asarray(inputs["W_hh"], dtype=np.float32)     # (H, H)
    b_sum = (np.asarray(inputs["b_ih"], dtype=np.float32)
             + np.asarray(inputs["b_hh"], dtype=np.float32))
    wq = np.asarray(inputs["Wq"], dtype=np.float32)
    wk = np.asarray(inputs["Wk"], dtype=np.float32)
    v_at = np.asarray(inputs["v_attn"], dtype=np.float32)
    fc_w = np.asarray(inputs["fc_W"], dtype=np.float32)     # (V, 2H)
    fc_b = np.asarray(inputs["fc_b"], dtype=np.float32)

    shared = {
        "emb": emb,
        "w_ihT": np.ascontiguousarray(w_ih.T),
        "w_hhT": np.ascontiguousarray(w_hh.T),
        "wqT": np.ascontiguousarray(wq.T),
        "wkT": np.ascontiguousarray(wk.T),
        "v_at": np.ascontiguousarray(v_at.reshape(H, 1)),
        "fc_w1T": np.ascontiguousarray(fc_w[:, :H].T),
        "fc_w2T": np.ascontiguousarray(fc_w[:, H:].T),
        "fc_bd": np.ascontiguousarray(fc_b.reshape(V, 1)),
        "b_sum": np.ascontiguousarray(b_sum.reshape(H, 1)),
    }
    in_maps = []
    for b in range(B):
        m = dict(shared)
        m["x_ids"] = np.ascontiguousarray(x[b].reshape(T, 1))
        m["h0c"] = np.ascontiguousarray(h0[0, b].reshape(H, 1))
        in_maps.append(m)
    return in_maps


def kernel(**inputs):
    nc = _get_nc()
    in_maps = make_in_maps(inputs)
    res = run_bass_kernel_spmd(nc, in_maps, core_ids=list(range(N_CORES)))
    logits = np.stack([np.ascontiguousarray(res.results[b]["logits_vT"].T)
                       for b in range(B)])
    hidden = np.stack([res.results[b]["h_last"][:, 0] for b in range(B)])[None]
    return logits, hidden


# revision 7
# speedup vs baseline: 1.9961x; 1.9961x over previous
"""AttentionRNN Trainium2 kernel.

Model (per batch element, sizes hardcoded):
    e = emb[x]                                          # (T=256, E=64) gather
    h_t = tanh(W_ih e_t + b_ih + W_hh h_{t-1} + b_hh)   # serial scan, H=128
    qk = Wq @ rnn_out ; kk = Wk @ rnn_out               # (H, T)
    score[t,s] = v . tanh(qk[:,t] + kk[:,s]), causal s<=t
    attn = softmax_s(score) ; ctx[:,t] = sum_s attn[t,s] rnn_out[:,s]
    logits = fc_W @ [rnn_out; ctx] + fc_b               # (V=96, T)

Sharding: pure data-parallel — core b computes batch element b end-to-end.
Host only reshapes/transposes/casts weights (layout prep) and stacks outputs.

Device design notes:
  * All matmul operands are fp16: fp32 matmuls lower to two LDWEIGHTS+
    MATMUL passes (~700ns even at N=1) which saturates the PE; fp16 is a
    single pass with fast weight load. Accumulation stays fp32 in PSUM,
    and the additive inputs that control accuracy (U = W_ih e + biases,
    softmax denominators, logits bias) stay fp32.
  * rnn_out, qk, kk, U live as [H=128 partitions, T=256 free].
  * Scan: per step one fp16 matmul (N=1) + one ACT tanh whose
    per-partition fp32 bias is the precomputed U column.
  * Pair tensor A[h, (t_chunk, s)] = qk[:,t]+kk[:,s] is built by a single
    DVE tensor_tensor add using broadcast access patterns, tanh'd on ACT.
    Key range is causally trimmed per 16-query chunk.
  * v-dot: per query, matmul with the pair slice as the *stationary*
    operand (lhsT [h, s_tile]) and v as the moving one -> scores land
    directly as [s, t] PSUM tiles (the lhsT free dim becomes partitions).
  * Softmax skips max subtraction: |score| <= ||v||_1 ~ 9, exp is safe
    and matches the reference's stabilized softmax to rounding.
    Causal mask = zero-fill after exp (affine_select), so masked keys drop
    out of both the denominator and the context sum. Diagonal score tiles
    are pre-zeroed so trimmed (never-written) PSUM regions can't produce
    NaN/Inf out of exp.
  * Denominator: ones-vector matmul over s -> [1, T] fp32 psum, copied to
    SBUF, moved to [t, 1] via a K=1 matmul, then reciprocal. Context is
    computed transposed (ctxT[t, h], attn as lhsT) so the normalization is
    a per-partition tensor_scalar multiply.
"""

import numpy as np

import concourse.bacc as bacc
import concourse.bass as bass
import concourse.tile as tile
from concourse import mybir
from concourse.bass_utils import run_bass_kernel_spmd
from concourse.masks import make_identity

B, T, V, E, H = 8, 256, 96, 64, 128
N_CORES = 8
FP = mybir.dt.float32
HP = mybir.dt.float16
AF = mybir.ActivationFunctionType
OP = mybir.AluOpType
AX = mybir.AxisListType

CQ = 16  # queries per pair-chunk
NCH = 128 // CQ


def _bcast_mid(ap2d, n):
    """[P, s] -> [P, (broadcast n), s]"""
    return bass.AP(tensor=ap2d.tensor, offset=ap2d.offset,
                   ap=[ap2d.ap[0], [0, n], ap2d.ap[1]])


def _bcast_inner(ap2d, n):
    """[P, c] -> [P, c, (broadcast n)]"""
    return bass.AP(tensor=ap2d.tensor, offset=ap2d.offset,
                   ap=[ap2d.ap[0], ap2d.ap[1], [0, n]])


def build_kernel():
    nc = bacc.Bacc()

    # Per-core inputs (weights replicated, x/h0 sharded by batch).
    x_ids = nc.dram_tensor("x_ids", [T, 1], mybir.dt.int32, kind="ExternalInput")
    h0c = nc.dram_tensor("h0c", [H, 1], HP, kind="ExternalInput")
    emb = nc.dram_tensor("emb", [V, E], FP, kind="ExternalInput")
    w_ihT = nc.dram_tensor("w_ihT", [E, H], HP, kind="ExternalInput")
    w_hhT = nc.dram_tensor("w_hhT", [H, H], HP, kind="ExternalInput")
    wqT = nc.dram_tensor("wqT", [H, H], HP, kind="ExternalInput")
    wkT = nc.dram_tensor("wkT", [H, H], HP, kind="ExternalInput")
    v_at = nc.dram_tensor("v_at", [H, 1], HP, kind="ExternalInput")
    fc_w1T = nc.dram_tensor("fc_w1T", [H, V], HP, kind="ExternalInput")
    fc_w2T = nc.dram_tensor("fc_w2T", [H, V], HP, kind="ExternalInput")
    fc_bd = nc.dram_tensor("fc_bd", [V, 1], FP, kind="ExternalInput")
    b_sum = nc.dram_tensor("b_sum", [H, 1], FP, kind="ExternalInput")

    logits_vT = nc.dram_tensor("logits_vT", [V, T], FP, kind="ExternalOutput")
    h_last = nc.dram_tensor("h_last", [H, 1], FP, kind="ExternalOutput")

    with tile.TileContext(nc) as tc:
        with (
            tc.tile_pool(name="consts", bufs=1) as consts,
            tc.tile_pool(name="work", bufs=1) as work,
            tc.tile_pool(name="pairs", bufs=2) as pairs,
        ):
            # ---- load constants ----
            ident = consts.tile([128, 128], FP)
            make_identity(nc, ident[:])
            ident16 = consts.tile([128, 128], HP)
            make_identity(nc, ident16[:])
            ones16 = consts.tile([H, 1], HP)
            nc.vector.memset(ones16[:], 1.0)
            w_ihT_s = consts.tile([E, H], HP)
            nc.sync.dma_start(out=w_ihT_s[:], in_=w_ihT[:])
            w_hhT_s = consts.tile([H, H], HP)
            nc.sync.dma_start(out=w_hhT_s[:], in_=w_hhT[:])
            wqT_s = consts.tile([H, H], HP)
            nc.sync.dma_start(out=wqT_s[:], in_=wqT[:])
            wkT_s = consts.tile([H, H], HP)
            nc.sync.dma_start(out=wkT_s[:], in_=wkT[:])
            v_s = consts.tile([H, 1], HP)
            nc.sync.dma_start(out=v_s[:], in_=v_at[:])
            fc_w1T_s = consts.tile([H, V], HP)
            nc.sync.dma_start(out=fc_w1T_s[:], in_=fc_w1T[:])
            fc_w2T_s = consts.tile([H, V], HP)
            nc.sync.dma_start(out=fc_w2T_s[:], in_=fc_w2T[:])
            fc_b_s = consts.tile([V, 1], FP)
            nc.sync.dma_start(out=fc_b_s[:], in_=fc_bd[:])
            b_sum_s = consts.tile([H, 1], FP)
            nc.sync.dma_start(out=b_sum_s[:], in_=b_sum[:])
            h0_s = consts.tile([H, 1], HP)
            nc.sync.dma_start(out=h0_s[:], in_=h0c[:])
            idx0 = consts.tile([128, 1], mybir.dt.int32)
            nc.sync.dma_start(out=idx0[:], in_=x_ids[0:128, :])
            idx1 = consts.tile([128, 1], mybir.dt.int32)
            nc.sync.dma_start(out=idx1[:], in_=x_ids[128:256, :])

            # ---- embedding gather + transpose to eT [E, T] (fp16) ----
            eT = work.tile([E, T], HP)
            e0 = work.tile([128, E], FP)
            nc.gpsimd.indirect_dma_start(
                out=e0[:], out_offset=None, in_=emb[:],
                in_offset=bass.IndirectOffsetOnAxis(ap=idx0[:, :1], axis=0))
            e1 = work.tile([128, E], FP)
            nc.gpsimd.indirect_dma_start(
                out=e1[:], out_offset=None, in_=emb[:],
                in_offset=bass.IndirectOffsetOnAxis(ap=idx1[:, :1], axis=0))

            u_sb = work.tile([H, T], FP)
            rnn = work.tile([H, T], HP)

            with tc.tile_pool(name="ps1", bufs=2, space="PSUM") as ps1:
                eT_ps0 = ps1.tile([E, 128], FP, tag="eT_ps")
                nc.tensor.transpose(out=eT_ps0[:], in_=e0[:], identity=ident[:])
                nc.vector.tensor_copy(out=eT[:, 0:128], in_=eT_ps0[:])
                eT_ps1 = ps1.tile([E, 128], FP, tag="eT_ps")
                nc.tensor.transpose(out=eT_ps1[:], in_=e1[:], identity=ident[:])
                nc.vector.tensor_copy(out=eT[:, 128:256], in_=eT_ps1[:])

                # ---- U = W_ih @ e + (b_ih + b_hh), fp32 [H, T] ----
                u_ps = ps1.tile([H, T], FP, tag="u_ps")
                nc.tensor.matmul(out=u_ps[:], lhsT=w_ihT_s[:], rhs=eT[:],
                                 start=True, stop=True)
                nc.scalar.activation(out=u_sb[:], in_=u_ps[:], func=AF.Identity,
                                     bias=b_sum_s[:, 0:1], scale=1.0)

                # ---- serial RNN scan ----
                hprev = h0_s[:, 0:1]
                for t in range(T):
                    g_ps = ps1.tile([H, 1], FP, tag="g_ps")
                    nc.tensor.matmul(out=g_ps[:], lhsT=w_hhT_s[:], rhs=hprev,
                                     start=True, stop=True)
                    nc.scalar.activation(out=rnn[:, t:t + 1], in_=g_ps[:],
                                         func=AF.Tanh, bias=u_sb[:, t:t + 1],
                                         scale=1.0)
                    hprev = rnn[:, t:t + 1]

            h_last_f = work.tile([H, 1], FP)
            nc.vector.tensor_copy(out=h_last_f[:], in_=rnn[:, T - 1:T])
            nc.sync.dma_start(out=h_last[:], in_=h_last_f[:])

            # ---- qk / kk [H, T] fp16 ----
            qk = work.tile([H, T], HP)
            kk = work.tile([H, T], HP)
            with tc.tile_pool(name="ps2", bufs=1, space="PSUM") as ps2:
                qk_ps = ps2.tile([H, T], FP, tag="qk_ps")
                nc.tensor.matmul(out=qk_ps[:], lhsT=wqT_s[:], rhs=rnn[:],
                                 start=True, stop=True)
                nc.vector.tensor_copy(out=qk[:], in_=qk_ps[:])
                kk_ps = ps2.tile([H, T], FP, tag="kk_ps")
                nc.tensor.matmul(out=kk_ps[:], lhsT=wkT_s[:], rhs=rnn[:],
                                 start=True, stop=True)
                nc.vector.tensor_copy(out=kk[:], in_=kk_ps[:])

            # ---- attention: scores in [s, t] psum tiles, exp, mask, ctx ----
            # att[tb][sb]: fp16 attention weights exp(score) for queries
            # t in [tb*128,(tb+1)*128), keys s in [sb*128,(sb+1)*128).
            att = {k: work.tile([128, 128], HP, tag=f"att{k[0]}{k[1]}",
                                name=f"att{k[0]}{k[1]}")
                   for k in ((0, 0), (1, 0), (1, 1))}
            rnnT0 = work.tile([128, H], HP)
            rnnT1 = work.tile([128, H], HP)
            ctx = work.tile([H, T], HP)
            den_sb = work.tile([1, T], FP)
            rdenT = work.tile([128, 2], FP)
            ctxT0 = work.tile([128, H], HP)
            ctxT1 = work.tile([128, H], HP)

            with tc.tile_pool(name="ps3", bufs=1, space="PSUM") as ps3:
                # rnnT tiles (needed for ctx): rnnT[sb] = rnn[:, sb].T
                tpa = ps3.tile([128, 128], HP, tag="mm", bufs=2)
                nc.tensor.transpose(out=tpa[:], in_=rnn[:, 0:128],
                                    identity=ident16[:])
                nc.vector.tensor_copy(out=rnnT0[:], in_=tpa[:])
                tpb = ps3.tile([128, 128], HP, tag="mm", bufs=2)
                nc.tensor.transpose(out=tpb[:], in_=rnn[:, 128:256],
                                    identity=ident16[:])
                nc.vector.tensor_copy(out=rnnT1[:], in_=tpb[:])

                for tb in range(2):
                    st_ps = [ps3.tile([128, 128], FP, tag="st", bufs=2,
                                      name=f"st{tb}{sb}")
                             for sb in range(tb + 1)]
                    # the diagonal tile has never-written (causally trimmed)
                    # regions; zero it so exp() sees finite garbage only
                    nc.vector.memset(st_ps[tb][:], 0.0)
                    for c in range(NCH):
                        q0 = tb * 128 + c * CQ
                        s_hi = q0 + CQ          # keys 0..s_hi-1
                        pair = pairs.tile([128, CQ, s_hi], HP, tag="pair")
                        nc.vector.tensor_tensor(
                            out=pair[:],
                            in0=_bcast_mid(kk[:, 0:s_hi], CQ),
                            in1=_bcast_inner(qk[:, q0:q0 + CQ], s_hi),
                            op=OP.add)
                        nc.scalar.activation(out=pair[:], in_=pair[:],
                                             func=AF.Tanh)
                        for j in range(CQ):
                            tcol = c * CQ + j
                            m0 = min(128, s_hi)
                            nc.tensor.matmul(
                                out=st_ps[0][0:m0, tcol:tcol + 1],
                                lhsT=pair[:, j, 0:m0],
                                rhs=v_s[:],
                                start=True, stop=True)
                            if tb == 1:
                                m1 = s_hi - 128
                                nc.tensor.matmul(
                                    out=st_ps[1][0:m1, tcol:tcol + 1],
                                    lhsT=pair[:, j, 128:s_hi],
                                    rhs=v_s[:],
                                    start=True, stop=True)
                    # exp (no max subtraction; |score| <= ||v||_1 ~ 9)
                    # + causal zero-fill on the diagonal block
                    for sb in range(tb + 1):
                        a_t = att[(tb, sb)]
                        nc.scalar.activation(out=a_t[:], in_=st_ps[sb][:],
                                             func=AF.Exp)
                        if sb == tb:  # keep where t - s >= 0
                            nc.gpsimd.affine_select(
                                out=a_t[:], in_=a_t[:], pattern=[[1, 128]],
                                compare_op=OP.is_ge, fill=0.0,
                                base=0, channel_multiplier=-1)

                # denominator den[t] = sum_s att[s, t]  -> [1, T] fp32 psum
                den_ps = ps3.tile([1, T], FP, tag="den")
                for tb in range(2):
                    for sb in range(tb + 1):
                        nc.tensor.matmul(
                            out=den_ps[:, tb * 128:(tb + 1) * 128],
                            lhsT=ones16[:], rhs=att[(tb, sb)][:],
                            start=(sb == 0), stop=(sb == tb))
                nc.vector.tensor_copy(out=den_sb[:], in_=den_ps[:])

                # denT via K=1 matmuls: out[t, 0] = den[0, t] (fp32 lhsT;
                # rhs = ident[0:1,0:1] == 1.0)
                denT_ps = ps3.tile([128, 2], FP, tag="mm", bufs=2)
                for tb in range(2):
                    nc.tensor.matmul(out=denT_ps[:, tb:tb + 1],
                                     lhsT=den_sb[:, tb * 128:(tb + 1) * 128],
                                     rhs=ident[0:1, 0:1],
                                     start=True, stop=True)
                nc.vector.reciprocal(out=rdenT[:], in_=denT_ps[:])

                # ctxT[t, h] = sum_s att[s, t] * rnnT[s, h], then scale by 1/den
                for tb, ctxT in ((0, ctxT0), (1, ctxT1)):
                    cps = ps3.tile([128, H], FP, tag="mm", bufs=2,
                                   name=f"cps{tb}")
                    for sb in range(tb + 1):
                        nc.tensor.matmul(
                            out=cps[:],
                            lhsT=att[(tb, sb)][:],
                            rhs=(rnnT0 if sb == 0 else rnnT1)[:],
                            start=(sb == 0), stop=(sb == tb))
                    nc.vector.tensor_scalar_mul(out=ctxT[:], in0=cps[:],
                                                scalar1=rdenT[:, tb:tb + 1])

                # ctx[h, t] = ctxT.T
                for tb, ctxT in ((0, ctxT0), (1, ctxT1)):
                    ctp = ps3.tile([H, 128], HP, tag="mm", bufs=2,
                                   name=f"ctp{tb}")
                    nc.tensor.transpose(out=ctp[:], in_=ctxT[:],
                                        identity=ident16[:])
                    nc.vector.tensor_copy(out=ctx[:, tb * 128:(tb + 1) * 128],
                                          in_=ctp[:])

                # ---- FC: logits = fc_W1 @ rnn + fc_W2 @ ctx + fc_b ----
                lg_ps = ps3.tile([V, T], FP, tag="lg")
                nc.tensor.matmul(out=lg_ps[:], lhsT=fc_w1T_s[:], rhs=rnn[:],
                                 start=True, stop=False)
                nc.tensor.matmul(out=lg_ps[:], lhsT=fc_w2T_s[:], rhs=ctx[:],
                                 start=False, stop=True)
                lg = work.tile([V, T], FP)
                nc.scalar.activation(out=lg[:], in_=lg_ps[:], func=AF.Identity,
                                     bias=fc_b_s[:, 0:1], scale=1.0)
                nc.sync.dma_start(out=logits_vT[:], in_=lg[:])

    nc.compile()
    return nc


_NC = None


def _get_nc():
    global _NC
    if _NC is None:
        _NC = build_kernel()
    return _NC


def make_in_maps(inputs):
    x = np.asarray(inputs["x"]).astype(np.int32)            # (B, T)
    h0 = np.asarray(inputs["h0"], dtype=np.float32)         # (1, B, H)
    emb = np.ascontiguousarray(np.asarray(inputs["emb"], dtype=np.float32))
    w_ih = np.asarray(inputs["W_ih"], dtype=np.float32)     # (H, E)
    w_hh = np.asarray(inputs["W_hh"], dtype=np.float32)     # (H, H)
    b_sum = (np.asarray(inputs["b_ih"], dtype=np.float32)
             + np.asarray(inputs["b_hh"], dtype=np.float32))
    wq = np.asarray(inputs["Wq"], dtype=np.float32)
    wk = np.asarray(inputs["Wk"], dtype=np.float32)
    v_at = np.asarray(inputs["v_attn"], dtype=np.float32)
    fc_w = np.asarray(inputs["fc_W"], dtype=np.float32)     # (V, 2H)
    fc_b = np.asarray(inputs["fc_b"], dtype=np.float32)

    f16 = np.float16
    shared = {
        "emb": emb,
        "w_ihT": np.ascontiguousarray(w_ih.T).astype(f16),
        "w_hhT": np.ascontiguousarray(w_hh.T).astype(f16),
        "wqT": np.ascontiguousarray(wq.T).astype(f16),
        "wkT": np.ascontiguousarray(wk.T).astype(f16),
        "v_at": np.ascontiguousarray(v_at.reshape(H, 1)).astype(f16),
        "fc_w1T": np.ascontiguousarray(fc_w[:, :H].T).astype(f16),
        "fc_w2T": np.ascontiguousarray(fc_w[:, H:].T).astype(f16),
        "fc_bd": np.ascontiguousarray(fc_b.reshape(V, 1)),
        "b_sum": np.ascontiguousarray(b_sum.reshape(H, 1)),
    }
    in_maps = []
    for b in range(B):
        m = dict(shared)
        m["x_ids"] = np.ascontiguousarray(x[b].reshape(T, 1))
        m["h0c"] = np.ascontiguousarray(h0[0, b].reshape(H, 1)).astype(f16)
        in_maps.append(m)
    return in_maps


def kernel(**inputs):
    nc = _get_nc()
    in_maps = make_in_maps(inputs)
    res = run_bass_kernel_spmd(nc, in_maps, core_ids=list(range(N_CORES)))
    logits = np.stack([np.ascontiguousarray(res.results[b]["logits_vT"].T)
                       for b in range(B)])
    hidden = np.stack([res.results[b]["h_last"][:, 0] for b in range(B)])[None]
    return logits, hidden
